# revision 4
# baseline (speedup 1.0000x reference)
"""Block-sparse linear kernel for Trainium2 (8 NeuronCores).

Computes: mask = mean|x| per 64x64 block > 0.798; out = (x*mask) @ weight
for x [4096,4096] f32, weight [4096,11008] f32 -> out [4096,11008] f32.

Strategy:
- Weight column-sharded across 8 cores (1376 cols each); x replicated.
  Each core computes its output shard; host concatenates. No collectives.
- Block sparsity (~48% live) exploited by streaming ONLY live 64x64
  x-blocks. The PE array runs in 2x2 tiled mode (four independent 64x64
  quadrants via matmul tile_position): quadrant (r, c) contracts
  parity-r k-blocks for the m-block currently owned by column-half c.
  Every streamed block is live - no pair-union padding (the previous
  scheme paired m-blocks on a 128-wide lhsT and streamed the union of
  their live sets, ~36% waste).
- Per m-block: even-parity k-blocks accumulate in PSUM pool (0,c), odd
  in (1,c); k-block parity (which SBUF half of the W tile stores it) is
  chosen host-side to minimize sum_m |#even_m - #odd_m| (per-m row
  lockstep cost) with a balanced local search. m-blocks are split
  across the two column-halves to balance total load; the two columns
  advance independently (merged emission alternates them MM-by-MM so
  all four quadrants stay busy).
- Drain per (m, chunk): ACT copies the odd PSUM half to SBUF, DVE adds
  even+odd into a staging tile, DMA to out rows. PSUM: 4 pools x 2
  bank-tiles = all 8 banks (double-buffered chains per quadrant).
- bf16 inputs (fp32 PSUM accumulation); mask-specialized schedule
  recompiled per input (NEFF cache makes repeat calls fast).
"""

import numpy as np
import ml_dtypes

import concourse.bacc as bacc
import concourse.mybir as mybir
import concourse.tile as tile
from concourse.bass_utils import run_bass_kernel_spmd

M = 4096
K = 4096
N = 11008
B = 64            # sparsity block
NB = M // B       # 64 blocks per dim
NCORES = 8
NSH = N // NCORES  # 1376 output cols per core
THRES = 0.798
CHUNKS = [(0, 512), (512, 512), (1024, 352)]  # N-chunks per psum bank
G = 16            # stream tiles (64 cols each) per DMA group
BF16 = mybir.dt.bfloat16
F32 = mybir.dt.float32


def _block_mask(x):
    xb = np.abs(x.reshape(NB, B, NB, B))
    bm = xb.mean(axis=(1, 3), dtype=np.float64)
    return bm > THRES


def _parity_assign(mask):
    """Balanced parity s[NB] in {0,1} minimizing sum_m |E_m - O_m|.
    Deterministic local search."""
    Mi = mask.astype(np.int32)
    rng = np.random.default_rng(1234)
    best_sig, best_c = None, 1 << 30
    for _ in range(6):
        sig = np.array([1] * (NB // 2) + [-1] * (NB // 2))
        rng.shuffle(sig)
        improved = True
        while improved:
            improved = False
            d = Mi @ sig
            cur = np.abs(d).sum()
            pos = np.where(sig == 1)[0]
            neg = np.where(sig == -1)[0]
            bestswap, bestdelta = None, 0
            for i in pos:
                for j in neg:
                    nd = d - 2 * Mi[:, i] + 2 * Mi[:, j]
                    delta = np.abs(nd).sum() - cur
                    if delta < bestdelta:
                        bestdelta, bestswap = delta, (i, j)
            if bestswap is not None:
                i, j = bestswap
                sig[i], sig[j] = -1, 1
                improved = True
        c = np.abs(Mi @ sig).sum()
        if c < best_c:
            best_c, best_sig = c, sig.copy()
    return (best_sig == -1).astype(np.int8)  # 1 = odd (array rows 64:128)


def _schedule(mask, par):
    """Per-m parity queues, W tile layout, balanced column split, and a
    merged emission schedule (columns alternate MM-by-MM)."""
    evens = [b for b in range(NB) if par[b] == 0]
    odds = [b for b in range(NB) if par[b] == 1]
    assert len(evens) == len(odds) == NB // 2
    wloc = {}
    for t in range(NB // 2):
        wloc[evens[t]] = (t, 0)
        wloc[odds[t]] = (t, 1)

    qE = {m: [b for b in evens if mask[m, b]] for m in range(NB)}
    qO = {m: [b for b in odds if mask[m, b]] for m in range(NB)}
    steps = {m: max(len(qE[m]), len(qO[m])) for m in range(NB)}

    order = sorted(range(NB), key=lambda m: -steps[m])
    cols, loads = [[], []], [0, 0]
    for m in order:
        c = 0 if loads[0] <= loads[1] else 1
        cols[c].append(m)
        loads[c] += steps[m]

    # per-column event lists
    evs = []
    for c in (0, 1):
        ev = []
        for m in cols[c]:
            for ci in range(len(CHUNKS)):
                for s in range(steps[m]):
                    ops = []
                    for r, q in ((0, qE[m]), (1, qO[m])):
                        if s < len(q):
                            ops.append((r, q[s], s == 0, s == len(q) - 1))
                    ev.append(("mm", m, ci, ops))
                ev.append(("drain", m, ci))
            ev.append(("out", m))
        evs.append(ev)

    # merge alternately; assign stream positions in merged (runtime) order
    merged = []
    i = j = 0
    while i < len(evs[0]) or j < len(evs[1]):
        if i < len(evs[0]):
            merged.append((0, evs[0][i]))
            i += 1
        if j < len(evs[1]):
            merged.append((1, evs[1][j]))
            j += 1

    pos = {}
    p = [0, 0]
    sched = []
    for c, ev in merged:
        if ev[0] == "mm":
            _, m, ci, ops = ev
            ops2 = []
            for (r, b, st, sp) in ops:
                if ci == 0:
                    pos[(r, m, b)] = p[r]
                    p[r] += 1
                ops2.append((r, b, pos[(r, m, b)], st, sp))
            sched.append((c, ("mm", m, ci, ops2)))
        else:
            sched.append((c, ev))
    return sched, wloc, pos, max(p)


def _pack_stream(x, pos, total):
    """bf16 stream [128, L*64]: row-half r partition range holds the
    transposed live x-blocks of parity r in consumption order."""
    L = max(G, ((total + G - 1) // G) * G)
    xs = np.zeros((128, L * 64), dtype=np.float32)
    for (r, m, b), pp in pos.items():
        blk = x[m * B:(m + 1) * B, b * B:(b + 1) * B]
        xs[64 * r:64 * r + 64, pp * 64:(pp + 1) * 64] = blk.T
    return xs.astype(ml_dtypes.bfloat16), L


def _w_row_index(wloc):
    idx = np.empty(K, dtype=np.int64)
    for b, (t, r) in wloc.items():
        idx[128 * t + 64 * r: 128 * t + 64 * r + 64] = np.arange(b * B, (b + 1) * B)
    return idx


def _build(sched, wloc, L, reps=1):
    nc = bacc.Bacc()
    xs_d = nc.declare_dram_parameter("xs", [128, L * 64], BF16, isOutput=False)
    w_d = nc.declare_dram_parameter("w", [K, NSH], BF16, isOutput=False)
    out_d = nc.declare_dram_parameter("out", [M, NSH], F32, isOutput=True)

    with tile.TileContext(nc) as tc:
        with (
            tc.tile_pool(name="wp", bufs=1) as wp,
            tc.tile_pool(name="xp", bufs=8) as xp,
            tc.tile_pool(name="dp", bufs=4) as dp,
            tc.tile_pool(name="sg0", bufs=3) as sg0,
            tc.tile_pool(name="sg1", bufs=3) as sg1,
            tc.tile_pool(name="p00", bufs=2, space="PSUM") as p00,
            tc.tile_pool(name="p10", bufs=2, space="PSUM") as p10,
            tc.tile_pool(name="p01", bufs=2, space="PSUM") as p01,
            tc.tile_pool(name="p11", bufs=2, space="PSUM") as p11,
        ):
            pools = {(0, 0): p00, (1, 0): p10, (0, 1): p01, (1, 1): p11}
            spools = {0: sg0, 1: sg1}
            wts = []
            for t in range(NB // 2):
                wt = wp.tile([128, NSH], BF16, tag=f"w{t}")
                nc.sync.dma_start(wt[:], w_d[128 * t:128 * (t + 1), :])
                wts.append(wt)

            for _ in range(reps):
                xg_tiles = {}
                cur_ps = {}
                cur_stage = {}
                for c, ev in sched:
                    if ev[0] == "mm":
                        _, m, ci, ops = ev
                        c0, cw = CHUNKS[ci]
                        for (r, b, pp, st, sp) in ops:
                            g = pp // G
                            if g not in xg_tiles:
                                xg = xp.tile([128, G * 64], BF16, tag="xg")
                                nc.sync.dma_start(
                                    xg[:], xs_d[:, g * G * 64:(g + 1) * G * 64])
                                xg_tiles[g] = xg
                            xg = xg_tiles[g]
                            if (c, r) not in cur_ps:
                                cur_ps[(c, r)] = pools[(r, c)].tile(
                                    [128, 512], F32, tag=f"ps{r}{c}",
                                    name=f"ps{r}{c}")
                            ps = cur_ps[(c, r)]
                            t, rr = wloc[b]
                            assert rr == r
                            off = (pp % G) * 64
                            nc.tensor.matmul(
                                ps[64 * c:64 * c + 64, :cw],
                                lhsT=xg[64 * r:64 * r + 64, off:off + 64],
                                rhs=wts[t][64 * r:64 * r + 64, c0:c0 + cw],
                                start=st,
                                stop=sp,
                                tile_position=(64 * r, 64 * c),
                                skip_group_check=True,
                            )
                    elif ev[0] == "drain":
                        _, m, ci = ev
                        c0, cw = CHUNKS[ci]
                        if c not in cur_stage:
                            cur_stage[c] = spools[c].tile(
                                [128, NSH], F32, tag=f"stage{c}",
                                name=f"stage{c}")
                        dst = cur_stage[c][64 * c:64 * c + 64, c0:c0 + cw]
                        pe = cur_ps.pop((c, 0), None)
                        po = cur_ps.pop((c, 1), None)
                        sl = slice(64 * c, 64 * c + 64)
                        if pe is not None and po is not None:
                            tmp = dp.tile([128, 512], F32, tag=f"tmp{c}")
                            nc.scalar.copy(tmp[sl, :cw], po[sl, :cw])
                            nc.vector.tensor_tensor(
                                dst, pe[sl, :cw], tmp[sl, :cw],
                                mybir.AluOpType.add)
                        elif pe is not None:
                            nc.vector.tensor_copy(dst, pe[sl, :cw])
                        elif po is not None:
                            nc.vector.tensor_copy(dst, po[sl, :cw])
                        else:
                            nc.vector.memset(dst, 0.0)
                    else:  # out
                        _, m = ev
                        stg = cur_stage.pop(c)
                        nc.sync.dma_start(
                            out_d[m * B:(m + 1) * B, :],
                            stg[64 * c:64 * c + 64, :])
    nc.compile()
    return nc


def _prepare(x, weight, reps=1):
    x = np.ascontiguousarray(np.asarray(x, dtype=np.float32))
    weight = np.ascontiguousarray(np.asarray(weight, dtype=np.float32))
    mask = _block_mask(x)
    par = _parity_assign(mask)
    sched, wloc, pos, total = _schedule(mask, par)
    xs, L = _pack_stream(x, pos, total)
    widx = _w_row_index(wloc)
    wperm = weight[widx].astype(ml_dtypes.bfloat16)
    in_maps = [
        {"xs": xs, "w": np.ascontiguousarray(wperm[:, c * NSH:(c + 1) * NSH])}
        for c in range(NCORES)
    ]
    nc = _build(sched, wloc, L, reps=reps)
    return nc, in_maps


def kernel(x, weight):
    nc, in_maps = _prepare(x, weight)
    res = run_bass_kernel_spmd(nc, in_maps, core_ids=list(range(NCORES)))
    out = np.concatenate([res.results[c]["out"] for c in range(NCORES)], axis=1)
    return np.ascontiguousarray(out)


# revision 8
# speedup vs baseline: 2.9681x; 2.9681x over previous
"""Block-sparse linear kernel for Trainium2 (8 NeuronCores).

Computes: mask = mean|x| per 64x64 block > 0.798; out = (x*mask) @ weight
for x [4096,4096] f32, weight [4096,11008] f32 -> out [4096,11008] f32.

Strategy:
- Weight column-sharded across 8 cores (1376 cols each); x replicated.
  Each core computes its output shard; host concatenates. No collectives.
- Block sparsity (~48% live) exploited by streaming ONLY live 64x64
  x-blocks. The PE array runs in 2x2 tiled mode (four independent 64x64
  quadrants via matmul tile_position): quadrant (r, c) contracts
  parity-r k-blocks for the m-block currently owned by column-half c.
  Every streamed block is live - no pair-union padding (the previous
  scheme paired m-blocks on a 128-wide lhsT and streamed the union of
  their live sets, ~36% waste).
- Per m-block: even-parity k-blocks accumulate in PSUM pool (0,c), odd
  in (1,c); k-block parity (which SBUF half of the W tile stores it) is
  chosen host-side to minimize sum_m |#even_m - #odd_m| (per-m row
  lockstep cost) with a balanced local search. m-blocks are split
  across the two column-halves to balance total load; the two columns
  advance independently (merged emission alternates them MM-by-MM so
  all four quadrants stay busy).
- Drain per (m, chunk): ACT copies the odd PSUM half to SBUF, DVE adds
  even+odd into a staging tile, DMA to out rows. PSUM: 4 pools x 2
  bank-tiles = all 8 banks (double-buffered chains per quadrant).
- bf16 inputs (fp32 PSUM accumulation); mask-specialized schedule
  recompiled per input (NEFF cache makes repeat calls fast).
"""

import numpy as np
import ml_dtypes

import concourse.bacc as bacc
import concourse.mybir as mybir
import concourse.tile as tile
from concourse.bass_utils import run_bass_kernel_spmd

M = 4096
K = 4096
N = 11008
B = 64            # sparsity block
NB = M // B       # 64 blocks per dim
NCORES = 8
NSH = N // NCORES  # 1376 output cols per core
THRES = 0.798
CHUNKS = [(0, 512), (512, 512), (1024, 352)]  # N-chunks per psum bank
G = 16            # stream tiles (64 cols each) per DMA group
BF16 = mybir.dt.bfloat16
F32 = mybir.dt.float32


def _block_mask(x):
    xb = np.abs(x.reshape(NB, B, NB, B))
    bm = xb.mean(axis=(1, 3), dtype=np.float64)
    return bm > THRES


def _parity_assign(mask):
    """Balanced parity s[NB] in {0,1}. Primary objective: global
    |sum E_m - sum O_m| (per-quadrant load balance under row drift);
    secondary: sum_m |E_m - O_m| (bounds how far rows drift apart).
    Deterministic local search."""
    Mi = mask.astype(np.int32)
    rng = np.random.default_rng(1234)

    def cost(sig):
        d = Mi @ sig
        return abs(int(d.sum())) * 64 + int(np.abs(d).sum())

    best_sig, best_c = None, 1 << 30
    for _ in range(6):
        sig = np.array([1] * (NB // 2) + [-1] * (NB // 2))
        rng.shuffle(sig)
        improved = True
        while improved:
            improved = False
            cur = cost(sig)
            pos = np.where(sig == 1)[0]
            neg = np.where(sig == -1)[0]
            bestswap, bestc = None, cur
            for i in pos:
                for j in neg:
                    sig[i], sig[j] = -1, 1
                    c = cost(sig)
                    sig[i], sig[j] = 1, -1
                    if c < bestc:
                        bestc, bestswap = c, (i, j)
            if bestswap is not None:
                i, j = bestswap
                sig[i], sig[j] = -1, 1
                improved = True
        c = cost(sig)
        if c < best_c:
            best_c, best_sig = c, sig.copy()
    return (best_sig == -1).astype(np.int8)  # 1 = odd (array rows 64:128)


def _schedule(mask, par):
    """Per-m parity queues, W tile layout, balanced column split, and a
    merged emission schedule. The two row-quadrants of a column drift
    independently across m-boundaries: when one parity's queue for the
    current m ends, that quadrant starts the next m's queue instead of
    idling through the other parity's remaining slots (the drain of
    (m, chunk) joins the two rows via its data dependency)."""
    evens = [b for b in range(NB) if par[b] == 0]
    odds = [b for b in range(NB) if par[b] == 1]
    assert len(evens) == len(odds) == NB // 2
    wloc = {}
    for t in range(NB // 2):
        wloc[evens[t]] = (t, 0)
        wloc[odds[t]] = (t, 1)

    qE = {m: [b for b in evens if mask[m, b]] for m in range(NB)}
    qO = {m: [b for b in odds if mask[m, b]] for m in range(NB)}
    tot = {m: len(qE[m]) + len(qO[m]) for m in range(NB)}

    # col split minimizing the max of the four quadrant loads
    # (sum of E / sum of O per column), greedy + pairwise swaps
    order = sorted(range(NB), key=lambda m: -tot[m])
    cols = [[], []]
    le, lo = [0, 0], [0, 0]
    for m in order:
        c = 0 if max(le[0] + len(qE[m]), lo[0] + len(qO[m])) <= \
            max(le[1] + len(qE[m]), lo[1] + len(qO[m])) else 1
        cols[c].append(m)
        le[c] += len(qE[m])
        lo[c] += len(qO[m])

    def qmax():
        return max(le[0], lo[0], le[1], lo[1])

    improved = True
    while improved:
        improved = False
        for a in list(cols[0]):
            for b2 in list(cols[1]):
                cur = qmax()
                dea, doa = len(qE[a]), len(qO[a])
                deb, dob = len(qE[b2]), len(qO[b2])
                le[0] += deb - dea
                lo[0] += dob - doa
                le[1] += dea - deb
                lo[1] += doa - dob
                if qmax() < cur:
                    cols[0].remove(a)
                    cols[1].remove(b2)
                    cols[0].append(b2)
                    cols[1].append(a)
                    improved = True
                    break
                le[0] -= deb - dea
                lo[0] -= dob - doa
                le[1] -= dea - deb
                lo[1] -= doa - dob
            if improved:
                break

    # order within a column: keep the E-O running imbalance near zero so
    # the rows never drift far apart (psum double-buffering absorbs ~1
    # round of drift)
    for c in (0, 1):
        rest = cols[c][:]
        out_order = []
        run = 0
        while rest:
            pick = min(rest, key=lambda m: abs(run + len(qE[m]) - len(qO[m])))
            out_order.append(pick)
            run += len(qE[pick]) - len(qO[pick])
            rest.remove(pick)
        cols[c] = out_order

    # per-column rotation packets with independent row progress
    evs = []
    for c in (0, 1):
        seqs, ends = [], []
        for r, qmap in ((0, qE), (1, qO)):
            s, end = [], {}
            for m in cols[c]:
                q = qmap[m]
                for ci in range(len(CHUNKS)):
                    for si, b in enumerate(q):
                        s.append((m, ci, b, si == 0, si == len(q) - 1))
                    end[(m, ci)] = len(s)
            seqs.append(s)
            ends.append(end)
        drains = [(m, ci) for m in cols[c] for ci in range(len(CHUNKS))]
        packets = []
        i = [0, 0]
        dptr = 0
        while i[0] < len(seqs[0]) or i[1] < len(seqs[1]) or dptr < len(drains):
            pkt = []
            for r in (0, 1):
                if i[r] < len(seqs[r]):
                    m, ci, b, st, sp = seqs[r][i[r]]
                    pkt.append(("mm", r, m, ci, b, st, sp))
                    i[r] += 1
            while dptr < len(drains):
                m, ci = drains[dptr]
                if ends[0][(m, ci)] <= i[0] and ends[1][(m, ci)] <= i[1]:
                    pkt.append(("drain", m, ci))
                    if ci == len(CHUNKS) - 1:
                        pkt.append(("out", m))
                    dptr += 1
                else:
                    break
            packets.append(pkt)
        evs.append(packets)

    # merge columns packet-by-packet; assign stream positions in merged
    # (runtime) order
    pos = {}
    p = [0, 0]
    sched = []
    i = j = 0
    while i < len(evs[0]) or j < len(evs[1]):
        for c, lst, k in ((0, evs[0], i), (1, evs[1], j)):
            if k < len(lst):
                for ev in lst[k]:
                    if ev[0] == "mm":
                        _, r, m, ci, b, st, sp = ev
                        if ci == 0:
                            pos[(r, m, b)] = p[r]
                            p[r] += 1
                        sched.append(
                            (c, ("mm", r, m, ci, b, pos[(r, m, b)], st, sp)))
                    else:
                        sched.append((c, ev))
        i += 1
        j += 1
    return sched, wloc, pos, max(p)


def _pack_stream(x, pos, total):
    """bf16 stream [128, L*64]: row-half r partition range holds the
    transposed live x-blocks of parity r in consumption order."""
    L = max(G, ((total + G - 1) // G) * G)
    xs = np.zeros((128, L * 64), dtype=np.float32)
    for (r, m, b), pp in pos.items():
        blk = x[m * B:(m + 1) * B, b * B:(b + 1) * B]
        xs[64 * r:64 * r + 64, pp * 64:(pp + 1) * 64] = blk.T
    return xs.astype(ml_dtypes.bfloat16), L


def _w_row_index(wloc):
    idx = np.empty(K, dtype=np.int64)
    for b, (t, r) in wloc.items():
        idx[128 * t + 64 * r: 128 * t + 64 * r + 64] = np.arange(b * B, (b + 1) * B)
    return idx


def _build(sched, wloc, L, reps=1):
    nc = bacc.Bacc()
    xs_d = nc.declare_dram_parameter("xs", [128, L * 64], BF16, isOutput=False)
    w_d = nc.declare_dram_parameter("w", [K, NSH], BF16, isOutput=False)
    out_d = nc.declare_dram_parameter("out", [M, NSH], F32, isOutput=True)

    with tile.TileContext(nc) as tc:
        with (
            tc.tile_pool(name="wp", bufs=1) as wp,
            tc.tile_pool(name="xp", bufs=8) as xp,
            tc.tile_pool(name="dp", bufs=4) as dp,
            tc.tile_pool(name="sg0", bufs=3) as sg0,
            tc.tile_pool(name="sg1", bufs=3) as sg1,
            tc.tile_pool(name="p00", bufs=2, space="PSUM") as p00,
            tc.tile_pool(name="p10", bufs=2, space="PSUM") as p10,
            tc.tile_pool(name="p01", bufs=2, space="PSUM") as p01,
            tc.tile_pool(name="p11", bufs=2, space="PSUM") as p11,
        ):
            pools = {(0, 0): p00, (1, 0): p10, (0, 1): p01, (1, 1): p11}
            spools = {0: sg0, 1: sg1}
            wts = []
            for t in range(NB // 2):
                wt = wp.tile([128, NSH], BF16, tag=f"w{t}")
                nc.sync.dma_start(wt[:], w_d[128 * t:128 * (t + 1), :])
                wts.append(wt)

            for _ in range(reps):
                xg_tiles = {}
                cur_ps = {}
                cur_stage = {}
                for c, ev in sched:
                    if ev[0] == "mm":
                        _, r, m, ci, b, pp, st, sp = ev
                        c0, cw = CHUNKS[ci]
                        g = pp // G
                        if g not in xg_tiles:
                            xg = xp.tile([128, G * 64], BF16, tag="xg")
                            nc.sync.dma_start(
                                xg[:], xs_d[:, g * G * 64:(g + 1) * G * 64])
                            xg_tiles[g] = xg
                        xg = xg_tiles[g]
                        key = (c, r, m, ci)
                        if key not in cur_ps:
                            cur_ps[key] = pools[(r, c)].tile(
                                [128, 512], F32, tag=f"ps{r}{c}",
                                name=f"ps{r}{c}")
                        ps = cur_ps[key]
                        t, rr = wloc[b]
                        assert rr == r
                        off = (pp % G) * 64
                        nc.tensor.matmul(
                            ps[64 * c:64 * c + 64, :cw],
                            lhsT=xg[64 * r:64 * r + 64, off:off + 64],
                            rhs=wts[t][64 * r:64 * r + 64, c0:c0 + cw],
                            start=st,
                            stop=sp,
                            tile_position=(64 * r, 64 * c),
                            skip_group_check=True,
                        )
                    elif ev[0] == "drain":
                        _, m, ci = ev
                        c0, cw = CHUNKS[ci]
                        if c not in cur_stage:
                            cur_stage[c] = spools[c].tile(
                                [128, NSH], F32, tag=f"stage{c}",
                                name=f"stage{c}")
                        dst = cur_stage[c][64 * c:64 * c + 64, c0:c0 + cw]
                        pe = cur_ps.pop((c, 0, m, ci), None)
                        po = cur_ps.pop((c, 1, m, ci), None)
                        sl = slice(64 * c, 64 * c + 64)
                        if pe is not None and po is not None:
                            tmp = dp.tile([128, 512], F32, tag=f"tmp{c}")
                            nc.scalar.copy(tmp[sl, :cw], po[sl, :cw])
                            nc.vector.tensor_tensor(
                                dst, pe[sl, :cw], tmp[sl, :cw],
                                mybir.AluOpType.add)
                        elif pe is not None:
                            nc.vector.tensor_copy(dst, pe[sl, :cw])
                        elif po is not None:
                            nc.vector.tensor_copy(dst, po[sl, :cw])
                        else:
                            nc.vector.memset(dst, 0.0)
                    else:  # out
                        _, m = ev
                        stg = cur_stage.pop(c)
                        nc.sync.dma_start(
                            out_d[m * B:(m + 1) * B, :],
                            stg[64 * c:64 * c + 64, :])
    nc.compile()
    return nc


def _prepare(x, weight, reps=1):
    x = np.ascontiguousarray(np.asarray(x, dtype=np.float32))
    weight = np.ascontiguousarray(np.asarray(weight, dtype=np.float32))
    mask = _block_mask(x)
    par = _parity_assign(mask)
    sched, wloc, pos, total = _schedule(mask, par)
    xs, L = _pack_stream(x, pos, total)
    widx = _w_row_index(wloc)
    wperm = weight[widx].astype(ml_dtypes.bfloat16)
    in_maps = [
        {"xs": xs, "w": np.ascontiguousarray(wperm[:, c * NSH:(c + 1) * NSH])}
        for c in range(NCORES)
    ]
    nc = _build(sched, wloc, L, reps=reps)
    return nc, in_maps


def kernel(x, weight):
    nc, in_maps = _prepare(x, weight)
    res = run_bass_kernel_spmd(nc, in_maps, core_ids=list(range(NCORES)))
    out = np.concatenate([res.results[c]["out"] for c in range(NCORES)], axis=1)
    return np.ascontiguousarray(out)
